# revision 33
# baseline (speedup 1.0000x reference)
"""Multi-head attention TRN2 kernel (Bass/Tile), 8-core tensor-parallel.

Sharding: core c -> batch b=c//4, head group g=c%4 (4 heads = 256 features).
Host pre-transposes x and weight slices to bf16; device computes qT/kT
(features x tokens) and v in token-major [v(64)|ones(64)] blocks per head,
so each AV matmul emits the softmax denominator replicated across psum
partitions 64:128, row-aligned with the context rows.  Causal softmax uses
unnormalized exp (scalar engine) with per-chunk column trimming at the
diagonal; 1/denominator is exp(-ln(x)) on the scalar engine (ln and exp
share one activation table), then one vector multiply.  The output
projection partial goes back fp32; the host sums 4 partials per batch and
adds the bias.

All matmul operands are bf16 (fp32 psum accumulation): full-rate PE at any
free dim, FWL weight loads that hide behind the matmuls, and 2x vector ops.
Pipeline: per 512-token slice: projections -> attention (heads paired per
feature tile, AV of group gi-1 issued between QK and exp of gi) -> output
projection delayed one or more slices so its matmuls fill the
activation-bound attention tail and keep the PE clock gate warm.
"""

import numpy as np

B, S, D = 2, 2048, 1024
H, HD = 16, 64
NCORES = 8
HPC = 4              # heads per core
FPC = HPC * HD       # 256 features per core
NF = FPC // 128      # 2 feature tiles of 128
KC = D // 128        # 8 contraction chunks
NTS = S // 512       # 4 token slices == q-tiles
NTT = S // 128       # 16 token tiles of 128
SCALE = 1.0 / 8.0    # 1/sqrt(HD)

_CACHE = {}


def _legalize_waits(nc, mybir, max_waits=1):
    """Walrus codegen allows only 1 sync-wait slot on most TPB instructions.
    Hoist extra waits into same-engine NoOps inserted just before."""
    n_fixed = 0
    for _, bb_wrap in nc.bb_map.items():
        bb = bb_wrap.bb
        out = []
        changed = False
        for inst in list(bb.instructions):
            si = inst.sync_info
            if si is not None and si.on_wait and len(si.on_wait) > max_waits:
                for w in list(si.on_wait[:-max_waits]):
                    nop = mybir.InstNoOp(
                        name=f"I-lw-{nc.next_id()}", engine=inst.engine,
                        ins=[], outs=[],
                        sync_info=mybir.SyncInfo(on_wait=[w], on_update=[]),
                    )
                    nop.text_hint = "dep"
                    out.append(nop)
                si.on_wait = list(si.on_wait[-max_waits:])
                n_fixed += 1
                changed = True
            out.append(inst)
        if changed:
            bb.instructions = out
    return n_fixed


def _build():
    import concourse.bass as bass
    import concourse.mybir as mybir
    from concourse.tile import TileContext
    from concourse.masks import make_upper_triangular

    F32 = mybir.dt.float32
    BF16 = mybir.dt.bfloat16
    EXP = mybir.ActivationFunctionType.Exp
    LN = mybir.ActivationFunctionType.Ln
    MUL = mybir.AluOpType.mult

    nc = bass.Bass()
    xT = nc.dram_tensor("xT", [D, S], BF16, kind="ExternalInput")
    wqT = nc.dram_tensor("wqT", [D, FPC], BF16, kind="ExternalInput")
    wkT = nc.dram_tensor("wkT", [D, FPC], BF16, kind="ExternalInput")
    wvT = nc.dram_tensor("wvT", [D, FPC], BF16, kind="ExternalInput")
    woT = nc.dram_tensor("woT", [FPC, D], BF16, kind="ExternalInput")
    outp = nc.dram_tensor("outp", [S, D], BF16, kind="ExternalOutput")

    with TileContext(nc) as tc:
        with (
            tc.tile_pool(name="res", bufs=1) as res,
            tc.tile_pool(name="xp", bufs=2) as xp,
            tc.tile_pool(name="wk", bufs=4) as wkp,
            tc.tile_pool(name="osb", bufs=4) as osb,
            tc.tile_pool(name="pjps", bufs=1, space="PSUM") as pjps,
            tc.tile_pool(name="qkps", bufs=2, space="PSUM") as qkps,
            tc.tile_pool(name="smps", bufs=1, space="PSUM") as smps,
            tc.tile_pool(name="avps", bufs=1, space="PSUM") as avps,
        ):
            # ---- resident tensors -------------------------------------
            qT = [res.tile([128, S], BF16, name=f"qT{f}", tag=f"qT{f}")
                  for f in range(NF)]
            kT = [res.tile([128, S], BF16, name=f"kT{f}", tag=f"kT{f}")
                  for f in range(NF)]
            ctxT = [res.tile([128, S], BF16, name=f"ctxT{f}", tag=f"ctxT{f}")
                    for f in range(NF)]
            # v chunk layout: per head hh a [v(64) | ones(64)] block, so the
            # AV matmul emits the softmax denominator replicated over
            # partitions 64:128 of its psum bank.
            v_sb = [res.tile([128, 512], BF16, name=f"v{ck}", tag=f"v{ck}")
                    for ck in range(NTT)]
            woT_sb = [res.tile([128, D], BF16, name=f"wo{ic}", tag=f"wo{ic}")
                      for ic in range(NF)]

            # x chunks for the first token slice: loaded before weights so
            # the first projection chain starts as early as possible.
            def load_x(ts):
                xch = []
                for e in range(KC):
                    t = xp.tile([128, 512], BF16, name=f"x{e}", tag=f"x{e}")
                    nc.gpsimd.dma_start(
                        out=t,
                        in_=xT[e * 128:(e + 1) * 128,
                               ts * 512:(ts + 1) * 512])
                    xch.append(t)
                return xch

            # PE warm-up: a dense burst of dummy matmuls during the initial
            # DMA wait flips the HAM clock gate to full rate before the
            # first projection group lands.  Alternating column halves of
            # one psum tile avoids back-to-back WAW serialization.  The
            # memset runs on the vector engine so no DMA issue can delay it.
            warm_sb = res.tile([128, 512], BF16, name="warm")
            nc.vector.memset(warm_sb, 0.0)
            wps = pjps.tile([128, 512], F32, name="pqk", tag="sm")
            for wi in range(24):
                c0 = 256 * (wi % 2)
                nc.tensor.matmul(wps[:, c0:c0 + 256], warm_sb[:, 0:128],
                                 warm_sb[:, 0:256], start=True, stop=True)

            # Interleave the first x slice with the wq chunks so the first
            # projection group can start after just two DMAs.
            w_t = {}
            _dmae = [nc.sync, nc.gpsimd]

            def load_w(nm, dram, e, eng=None):
                t = wkp.tile([128, FPC], BF16, name=f"w{nm}{e}",
                             tag=f"w{nm}{e}", bufs=1)
                (eng or nc.sync).dma_start(
                    out=t, in_=dram[e * 128:(e + 1) * 128, :])
                w_t[nm, e] = t

            xch0 = []
            for e in range(KC):
                t = xp.tile([128, 512], BF16, name=f"x{e}", tag=f"x{e}")
                _dmae[e % 2].dma_start(
                    out=t, in_=xT[e * 128:(e + 1) * 128, 0:512])
                xch0.append(t)
                load_w("q", wqT, e, _dmae[(e + 1) % 2])
            for e in range(KC):
                load_w("k", wkT, e, _dmae[e % 2])
            for e in range(KC):
                load_w("v", wvT, e, _dmae[(e + 1) % 2])

            # constants
            ones_f = res.tile([128, 64], F32)
            nc.gpsimd.memset(ones_f, 1.0)
            mask_f = res.tile([128, 128], F32)
            make_upper_triangular(nc, mask_f, val=1.0, diag=True)
            mask_r = res.tile([128, 128], BF16)
            nc.vector.tensor_copy(mask_r, mask_f)
            for ck in range(NTT):
                v4 = v_sb[ck].rearrange("p (h c) -> p h c", c=128)
                for hh in range(HPC):
                    nc.vector.tensor_copy(v4[:, hh, 64:128], ones_f)

            for ic in range(NF):
                nc.sync.dma_start(
                    out=woT_sb[ic],
                    in_=woT[ic * 128:(ic + 1) * 128, :])

            # ---- pipelined slices -------------------------------------
            for ts in range(NTS):
                xch = xch0 if ts == 0 else load_x(ts)

                # projections for this slice
                for nm, dst in (("q", qT), ("k", kT)):
                    for f in range(NF):
                        ps = pjps.tile([128, 512], F32, name="pqk", tag="sm")
                        for e in range(KC):
                            nc.tensor.matmul(
                                ps, w_t[nm, e][:, f * 128:(f + 1) * 128],
                                xch[e], start=(e == 0), stop=(e == KC - 1))
                        nc.vector.tensor_copy(
                            dst[f][:, ts * 512:(ts + 1) * 512], ps)
                for tt in range(4):
                    ck = ts * 4 + tt
                    ps = pjps.tile([128, FPC], F32, name="pv", tag="sm")
                    for e in range(KC):
                        nc.tensor.matmul(
                            ps, xch[e][:, tt * 128:(tt + 1) * 128],
                            w_t["v", e], start=(e == 0), stop=(e == KC - 1))
                    v4 = v_sb[ck].rearrange("p (h c) -> p h c", c=128)
                    ps4 = ps.rearrange("p (g c) -> p g c", c=64)
                    nc.vector.tensor_copy(v4[:, :, 0:64], ps4)

                # attention for q-tile j == ts.  The two heads of each
                # feature tile f run as a row-tiled pair: head hl=0 on PE
                # rows 0:64 (tile_position (0,0)), hl=1 on rows 64:128
                # ((64,0)) — concurrent on the array, halving QK^T time.
                # AV matmuls of group gi-1 are issued between the QK pair
                # of gi and its exp so the PE stays busy during the
                # activation.
                j = ts
                sq0 = 512 * j
                ndiag = 4 * j  # first diagonal chunk index

                def qk_trim(ci):
                    """Leading fully-masked columns to skip."""
                    return max(128 * ci - sq0, 0)

                for f in range(NF):
                    av = avps.tile([128, 1024], F32, name="av", tag="av")
                    ag_prev = None

                    def av_mms(ag_pair, gi):
                        for slot in range(2):
                            ci = 2 * gi + slot
                            tr = qk_trim(ci)
                            for hl in range(2):
                                hh = 2 * f + hl
                                nc.tensor.matmul(
                                    av[:, 512 * hl + tr:512 * hl + 512],
                                    v_sb[ci][:, 128 * hh:128 * hh + 128],
                                    ag_pair[hl][:, slot * 512 + tr:
                                                 slot * 512 + 512],
                                    start=(ci == 0),
                                    stop=(ci == 4 * j + 3))

                    for gi in range(2 * j + 2):
                        qk_pair = [qkps.tile([128, 1024], F32, name="qk",
                                             tag="qk") for _ in range(2)]
                        for slot in range(2):
                            ci = 2 * gi + slot
                            tr = qk_trim(ci)
                            for hl in range(2):
                                r0 = 64 * hl
                                nc.tensor.matmul(
                                    qk_pair[hl][:, slot * 512 + tr:
                                                slot * 512 + 512],
                                    kT[f][r0:r0 + 64,
                                          ci * 128:(ci + 1) * 128],
                                    qT[f][r0:r0 + 64,
                                          sq0 + tr:sq0 + 512],
                                    start=True, stop=True)
                        if ag_prev is not None:
                            av_mms(ag_prev, gi - 1)
                        ag_pair = [osb.tile([128, 1024], BF16, name="ag",
                                            tag="ag", bufs=6)
                                   for _ in range(2)]
                        for hl in range(2):
                            qk, ag = qk_pair[hl], ag_pair[hl]
                            if gi < 2 * j:  # fully-valid chunks
                                nc.scalar.activation(ag, qk, EXP,
                                                     scale=SCALE)
                            else:
                                # diagonal: one exp from the first valid
                                # column onward (cols the av matmuls never
                                # read may hold garbage), then triangular
                                # masks per 128-chunk.
                                tr0 = qk_trim(2 * gi)
                                nc.scalar.activation(
                                    ag[:, tr0:1024], qk[:, tr0:1024],
                                    EXP, scale=SCALE)
                                for slot in range(2):
                                    ci = 2 * gi + slot
                                    dlt = 128 * ci - sq0
                                    w0 = slot * 512
                                    nc.vector.tensor_tensor(
                                        ag[:, w0 + dlt:w0 + dlt + 128],
                                        ag[:, w0 + dlt:w0 + dlt + 128],
                                        mask_r, MUL)
                        ag_prev = ag_pair
                    av_mms(ag_prev, 2 * j + 1)

                    # normalize: ctxT = av[0:64] * (1/av[64:128]); the ones
                    # half of each v block put the denominator in av rows
                    # 64:128 (replicated), row-aligned with ctx.  1/x is
                    # exp(-ln(x)) on the scalar engine: ln and exp share one
                    # activation table.  Both heads batched per call.
                    if ts == NTS - 1 and f == NF - 1:
                        # the kernel tail waits on this normalize; split it
                        # per head to shorten the critical chain
                        for hl in range(2):
                            r0, c0 = 64 * hl, 512 * hl
                            lns = osb.tile([64, 512], F32, name="lns",
                                           tag="lns")
                            nc.scalar.activation(
                                lns, av[64:128, c0:c0 + 512], LN)
                            rcp = osb.tile([64, 512], F32, name="rcp",
                                           tag="rcp")
                            nc.scalar.activation(rcp, lns, EXP, scale=-1.0)
                            nc.vector.tensor_tensor(
                                ctxT[f][r0:r0 + 64, sq0:sq0 + 512],
                                av[0:64, c0:c0 + 512], rcp, MUL)
                    else:
                        lns = osb.tile([64, 1024], F32, name="lns",
                                       tag="lns")
                        nc.scalar.activation(lns, av[64:128, :], LN)
                        rcp = osb.tile([64, 1024], F32, name="rcp",
                                       tag="rcp")
                        nc.scalar.activation(rcp, lns, EXP, scale=-1.0)
                        for hl in range(2):
                            r0 = 64 * hl
                            nc.vector.tensor_tensor(
                                ctxT[f][r0:r0 + 64, sq0:sq0 + 512],
                                av[0:64, 512 * hl:512 * hl + 512],
                                rcp[:, 512 * hl:512 * hl + 512], MUL)

                # output projection, delayed one slice: emitted after the
                # NEXT slice's attention so its matmuls act as PE filler
                # while that attention is activation-bound.  The last two
                # slices' projections also alternate into the (by then
                # idle) pjps bank for double-buffering.
                def out_proj(ots):
                    for tt in range(4 * ots, 4 * ots + 4):
                        for os_ in range(2):
                            pool = pjps if (ots > 0 and os_ == 1) else smps
                            po = pool.tile([128, 512], F32, name="po",
                                           tag="po" if pool is smps
                                           else "sm")
                            for ic in range(NF):
                                nc.tensor.matmul(
                                    po,
                                    ctxT[ic][:, tt * 128:(tt + 1) * 128],
                                    woT_sb[ic][:, os_ * 512:(os_ + 1) * 512],
                                    start=(ic == 0), stop=(ic == NF - 1))
                            so = osb.tile([128, 512], BF16, name="so",
                                          tag="so")
                            nc.vector.tensor_copy(so, po)
                            nc.sync.dma_start(
                                out=outp[tt * 128:(tt + 1) * 128,
                                         os_ * 512:(os_ + 1) * 512],
                                in_=so)

                # op0 fills attention(1); op1..3 are emitted last so they
                # act as tail filler while attention(2,3) is
                # activation-bound, keeping the PE dense enough to stay at
                # full clock.
                if ts == 1:
                    out_proj(0)
            for ots in (1, 2, 3):
                out_proj(ots)

    _legalize_waits(nc, mybir)
    return nc


def _prep_inputs(in_data, Wq, Wk, Wv, Wo):
    import ml_dtypes

    bf16 = ml_dtypes.bfloat16
    in_maps = []
    for c in range(NCORES):
        b, g = c // 4, c % 4
        sl = slice(g * FPC, (g + 1) * FPC)
        in_maps.append({
            "xT": np.ascontiguousarray(in_data[b].T).astype(bf16),
            "wqT": np.ascontiguousarray(Wq[sl, :].T).astype(bf16),
            "wkT": np.ascontiguousarray(Wk[sl, :].T).astype(bf16),
            "wvT": np.ascontiguousarray(Wv[sl, :].T).astype(bf16),
            "woT": np.ascontiguousarray(Wo[:, sl].T).astype(bf16),
        })
    return in_maps


def run(inputs, trace=False):
    from concourse.bass_utils import run_bass_kernel_spmd

    in_data = np.asarray(inputs["in_data"], dtype=np.float32)
    Wq = np.asarray(inputs["Wq"], dtype=np.float32)
    Wk = np.asarray(inputs["Wk"], dtype=np.float32)
    Wv = np.asarray(inputs["Wv"], dtype=np.float32)
    Wo = np.asarray(inputs["Wo"], dtype=np.float32)
    bo = np.asarray(inputs["bo"], dtype=np.float32)

    if "nc" not in _CACHE:
        _CACHE["nc"] = _build()
    nc = _CACHE["nc"]

    in_maps = _prep_inputs(in_data, Wq, Wk, Wv, Wo)
    kw = {}
    if trace:
        kw = dict(trace=True, trace_cores=list(range(NCORES)))
    res = run_bass_kernel_spmd(nc, in_maps, core_ids=list(range(NCORES)), **kw)

    out = np.zeros((B, S, D), dtype=np.float32)
    for c in range(NCORES):
        out[c // 4] += res.results[c]["outp"]
    out += bo[None, None, :]
    return out, res


def kernel(**inputs) -> np.ndarray:
    out, _ = run(inputs)
    return out



# revision 34
# speedup vs baseline: 1.0128x; 1.0128x over previous
"""Multi-head attention TRN2 kernel (Bass/Tile), 8-core tensor-parallel.

Sharding: core c -> batch b=c//4, head group g=c%4 (4 heads = 256 features).
Host pre-transposes x and weight slices to bf16; device computes qT/kT
(features x tokens) and v in token-major [v(64)|ones(64)] blocks per head,
so each AV matmul emits the softmax denominator replicated across psum
partitions 64:128, row-aligned with the context rows.  Causal softmax uses
unnormalized exp (scalar engine) with per-chunk column trimming at the
diagonal; 1/denominator is exp(-ln(x)) on the scalar engine (ln and exp
share one activation table), then one vector multiply.  The output
projection partial goes back fp32; the host sums 4 partials per batch and
adds the bias.

All matmul operands are bf16 (fp32 psum accumulation): full-rate PE at any
free dim, FWL weight loads that hide behind the matmuls, and 2x vector ops.
Pipeline: per 512-token slice: projections -> attention (heads paired per
feature tile, AV of group gi-1 issued between QK and exp of gi) -> output
projection delayed one or more slices so its matmuls fill the
activation-bound attention tail and keep the PE clock gate warm.
"""

import numpy as np

B, S, D = 2, 2048, 1024
H, HD = 16, 64
NCORES = 8
HPC = 4              # heads per core
FPC = HPC * HD       # 256 features per core
NF = FPC // 128      # 2 feature tiles of 128
KC = D // 128        # 8 contraction chunks
NTS = S // 512       # 4 token slices == q-tiles
NTT = S // 128       # 16 token tiles of 128
SCALE = 1.0 / 8.0    # 1/sqrt(HD)

_CACHE = {}


def _legalize_waits(nc, mybir, max_waits=1):
    """Walrus codegen allows only 1 sync-wait slot on most TPB instructions.
    Hoist extra waits into same-engine NoOps inserted just before."""
    n_fixed = 0
    for _, bb_wrap in nc.bb_map.items():
        bb = bb_wrap.bb
        out = []
        changed = False
        for inst in list(bb.instructions):
            si = inst.sync_info
            if si is not None and si.on_wait and len(si.on_wait) > max_waits:
                for w in list(si.on_wait[:-max_waits]):
                    nop = mybir.InstNoOp(
                        name=f"I-lw-{nc.next_id()}", engine=inst.engine,
                        ins=[], outs=[],
                        sync_info=mybir.SyncInfo(on_wait=[w], on_update=[]),
                    )
                    nop.text_hint = "dep"
                    out.append(nop)
                si.on_wait = list(si.on_wait[-max_waits:])
                n_fixed += 1
                changed = True
            out.append(inst)
        if changed:
            bb.instructions = out
    return n_fixed


def _build():
    import concourse.bass as bass
    import concourse.mybir as mybir
    from concourse.tile import TileContext
    from concourse.masks import make_upper_triangular

    F32 = mybir.dt.float32
    BF16 = mybir.dt.bfloat16
    EXP = mybir.ActivationFunctionType.Exp
    LN = mybir.ActivationFunctionType.Ln
    MUL = mybir.AluOpType.mult

    nc = bass.Bass()
    xT = nc.dram_tensor("xT", [D, S], BF16, kind="ExternalInput")
    wqT = nc.dram_tensor("wqT", [D, FPC], BF16, kind="ExternalInput")
    wkT = nc.dram_tensor("wkT", [D, FPC], BF16, kind="ExternalInput")
    wvT = nc.dram_tensor("wvT", [D, FPC], BF16, kind="ExternalInput")
    woT = nc.dram_tensor("woT", [FPC, D], BF16, kind="ExternalInput")
    outp = nc.dram_tensor("outp", [S, D], BF16, kind="ExternalOutput")

    with TileContext(nc) as tc:
        with (
            tc.tile_pool(name="res", bufs=1) as res,
            tc.tile_pool(name="xp", bufs=2) as xp,
            tc.tile_pool(name="wk", bufs=4) as wkp,
            tc.tile_pool(name="osb", bufs=4) as osb,
            tc.tile_pool(name="pjps", bufs=1, space="PSUM") as pjps,
            tc.tile_pool(name="qkps", bufs=2, space="PSUM") as qkps,
            tc.tile_pool(name="smps", bufs=1, space="PSUM") as smps,
            tc.tile_pool(name="avps", bufs=1, space="PSUM") as avps,
        ):
            # ---- resident tensors -------------------------------------
            qT = [res.tile([128, S], BF16, name=f"qT{f}", tag=f"qT{f}")
                  for f in range(NF)]
            kT = [res.tile([128, S], BF16, name=f"kT{f}", tag=f"kT{f}")
                  for f in range(NF)]
            ctxT = [res.tile([128, S], BF16, name=f"ctxT{f}", tag=f"ctxT{f}")
                    for f in range(NF)]
            # v chunk layout: per head hh a [v(64) | ones(64)] block, so the
            # AV matmul emits the softmax denominator replicated over
            # partitions 64:128 of its psum bank.
            v_sb = [res.tile([128, 512], BF16, name=f"v{ck}", tag=f"v{ck}")
                    for ck in range(NTT)]
            woT_sb = [res.tile([128, D], BF16, name=f"wo{ic}", tag=f"wo{ic}")
                      for ic in range(NF)]

            # x chunks for the first token slice: loaded before weights so
            # the first projection chain starts as early as possible.
            def load_x(ts):
                xch = []
                for e in range(KC):
                    t = xp.tile([128, 512], BF16, name=f"x{e}", tag=f"x{e}")
                    nc.sync.dma_start(
                        out=t,
                        in_=xT[e * 128:(e + 1) * 128,
                               ts * 512:(ts + 1) * 512])
                    xch.append(t)
                return xch

            # Interleave the first x slice with the wq chunks so the first
            # projection group can start after just two DMAs.
            w_t = {}

            def load_w(nm, dram, e):
                t = wkp.tile([128, FPC], BF16, name=f"w{nm}{e}",
                             tag=f"w{nm}{e}", bufs=1)
                nc.sync.dma_start(
                    out=t, in_=dram[e * 128:(e + 1) * 128, :])
                w_t[nm, e] = t

            xch0 = []
            for e in range(KC):
                t = xp.tile([128, 512], BF16, name=f"x{e}", tag=f"x{e}")
                nc.sync.dma_start(
                    out=t, in_=xT[e * 128:(e + 1) * 128, 0:512])
                xch0.append(t)
                load_w("q", wqT, e)
            for e in range(KC):
                load_w("k", wkT, e)
            for e in range(KC):
                load_w("v", wvT, e)

            # PE warm-up: a dense burst of dummy matmuls during the initial
            # DMA wait flips the HAM clock gate to full rate before the
            # first projection group lands.  Alternating column halves of
            # one psum tile avoids back-to-back WAW serialization.
            warm_sb = res.tile([128, 512], BF16, name="warm")
            nc.gpsimd.memset(warm_sb, 0.0)
            wps = pjps.tile([128, 512], F32, name="pqk", tag="sm")
            for wi in range(24):
                c0 = 256 * (wi % 2)
                nc.tensor.matmul(wps[:, c0:c0 + 256], warm_sb[:, 0:128],
                                 warm_sb[:, 0:256], start=True, stop=True)

            # constants
            ones_f = res.tile([128, 64], F32)
            nc.gpsimd.memset(ones_f, 1.0)
            mask_f = res.tile([128, 128], F32)
            make_upper_triangular(nc, mask_f, val=1.0, diag=True)
            mask_r = res.tile([128, 128], BF16)
            nc.vector.tensor_copy(mask_r, mask_f)
            for ck in range(NTT):
                v4 = v_sb[ck].rearrange("p (h c) -> p h c", c=128)
                for hh in range(HPC):
                    nc.vector.tensor_copy(v4[:, hh, 64:128], ones_f)

            for ic in range(NF):
                nc.sync.dma_start(
                    out=woT_sb[ic],
                    in_=woT[ic * 128:(ic + 1) * 128, :])

            # ---- pipelined slices -------------------------------------
            for ts in range(NTS):
                xch = xch0 if ts == 0 else load_x(ts)

                # projections for this slice
                for nm, dst in (("q", qT), ("k", kT)):
                    for f in range(NF):
                        ps = pjps.tile([128, 512], F32, name="pqk", tag="sm")
                        for e in range(KC):
                            nc.tensor.matmul(
                                ps, w_t[nm, e][:, f * 128:(f + 1) * 128],
                                xch[e], start=(e == 0), stop=(e == KC - 1))
                        nc.vector.tensor_copy(
                            dst[f][:, ts * 512:(ts + 1) * 512], ps)
                for tt in range(4):
                    ck = ts * 4 + tt
                    ps = pjps.tile([128, FPC], F32, name="pv", tag="sm")
                    for e in range(KC):
                        nc.tensor.matmul(
                            ps, xch[e][:, tt * 128:(tt + 1) * 128],
                            w_t["v", e], start=(e == 0), stop=(e == KC - 1))
                    v4 = v_sb[ck].rearrange("p (h c) -> p h c", c=128)
                    ps4 = ps.rearrange("p (g c) -> p g c", c=64)
                    nc.vector.tensor_copy(v4[:, :, 0:64], ps4)

                # attention for q-tile j == ts.  The two heads of each
                # feature tile f run as a row-tiled pair: head hl=0 on PE
                # rows 0:64 (tile_position (0,0)), hl=1 on rows 64:128
                # ((64,0)) — concurrent on the array, halving QK^T time.
                # AV matmuls of group gi-1 are issued between the QK pair
                # of gi and its exp so the PE stays busy during the
                # activation.
                j = ts
                sq0 = 512 * j
                ndiag = 4 * j  # first diagonal chunk index

                def qk_trim(ci):
                    """Leading fully-masked columns to skip."""
                    return max(128 * ci - sq0, 0)

                for f in range(NF):
                    av = avps.tile([128, 1024], F32, name="av", tag="av")
                    ag_prev = None

                    def av_mms(ag_pair, gi):
                        for slot in range(2):
                            ci = 2 * gi + slot
                            tr = qk_trim(ci)
                            for hl in range(2):
                                hh = 2 * f + hl
                                nc.tensor.matmul(
                                    av[:, 512 * hl + tr:512 * hl + 512],
                                    v_sb[ci][:, 128 * hh:128 * hh + 128],
                                    ag_pair[hl][:, slot * 512 + tr:
                                                 slot * 512 + 512],
                                    start=(ci == 0),
                                    stop=(ci == 4 * j + 3))

                    for gi in range(2 * j + 2):
                        qk_pair = [qkps.tile([128, 1024], F32, name="qk",
                                             tag="qk") for _ in range(2)]
                        for slot in range(2):
                            ci = 2 * gi + slot
                            tr = qk_trim(ci)
                            for hl in range(2):
                                r0 = 64 * hl
                                nc.tensor.matmul(
                                    qk_pair[hl][:, slot * 512 + tr:
                                                slot * 512 + 512],
                                    kT[f][r0:r0 + 64,
                                          ci * 128:(ci + 1) * 128],
                                    qT[f][r0:r0 + 64,
                                          sq0 + tr:sq0 + 512],
                                    start=True, stop=True)
                        if ag_prev is not None:
                            av_mms(ag_prev, gi - 1)
                        ag_pair = [osb.tile([128, 1024], BF16, name="ag",
                                            tag="ag", bufs=6)
                                   for _ in range(2)]
                        for hl in range(2):
                            qk, ag = qk_pair[hl], ag_pair[hl]
                            if gi < 2 * j:  # fully-valid chunks
                                nc.scalar.activation(ag, qk, EXP,
                                                     scale=SCALE)
                            else:
                                # diagonal: one exp from the first valid
                                # column onward (cols the av matmuls never
                                # read may hold garbage), then triangular
                                # masks per 128-chunk.
                                tr0 = qk_trim(2 * gi)
                                nc.scalar.activation(
                                    ag[:, tr0:1024], qk[:, tr0:1024],
                                    EXP, scale=SCALE)
                                for slot in range(2):
                                    ci = 2 * gi + slot
                                    dlt = 128 * ci - sq0
                                    w0 = slot * 512
                                    nc.vector.tensor_tensor(
                                        ag[:, w0 + dlt:w0 + dlt + 128],
                                        ag[:, w0 + dlt:w0 + dlt + 128],
                                        mask_r, MUL)
                        ag_prev = ag_pair
                    av_mms(ag_prev, 2 * j + 1)

                    # normalize: ctxT = av[0:64] * (1/av[64:128]); the ones
                    # half of each v block put the denominator in av rows
                    # 64:128 (replicated), row-aligned with ctx.  1/x is
                    # exp(-ln(x)) on the scalar engine: ln and exp share one
                    # activation table.  Both heads batched per call.
                    lns = osb.tile([64, 1024], F32, name="lns", tag="lns")
                    nc.scalar.activation(lns, av[64:128, :], LN)
                    rcp = osb.tile([64, 1024], F32, name="rcp", tag="rcp")
                    nc.scalar.activation(rcp, lns, EXP, scale=-1.0)
                    for hl in range(2):
                        r0 = 64 * hl
                        nc.vector.tensor_tensor(
                            ctxT[f][r0:r0 + 64, sq0:sq0 + 512],
                            av[0:64, 512 * hl:512 * hl + 512],
                            rcp[:, 512 * hl:512 * hl + 512], MUL)

                # output projection, delayed one slice: emitted after the
                # NEXT slice's attention so its matmuls act as PE filler
                # while that attention is activation-bound.  The last two
                # slices' projections also alternate into the (by then
                # idle) pjps bank for double-buffering.
                def out_proj(ots):
                    for tt in range(4 * ots, 4 * ots + 4):
                        for os_ in range(2):
                            pool = pjps if (ots > 0 and os_ == 1) else smps
                            po = pool.tile([128, 512], F32, name="po",
                                           tag="po" if pool is smps
                                           else "sm")
                            for ic in range(NF):
                                nc.tensor.matmul(
                                    po,
                                    ctxT[ic][:, tt * 128:(tt + 1) * 128],
                                    woT_sb[ic][:, os_ * 512:(os_ + 1) * 512],
                                    start=(ic == 0), stop=(ic == NF - 1))
                            so = osb.tile([128, 512], BF16, name="so",
                                          tag="so")
                            nc.vector.tensor_copy(so, po)
                            nc.sync.dma_start(
                                out=outp[tt * 128:(tt + 1) * 128,
                                         os_ * 512:(os_ + 1) * 512],
                                in_=so)

                # op0 fills attention(1); op1..3 are emitted last so they
                # act as tail filler while attention(2,3) is
                # activation-bound, keeping the PE dense enough to stay at
                # full clock.
                if ts == 1:
                    out_proj(0)
            for ots in (1, 2, 3):
                out_proj(ots)

    _legalize_waits(nc, mybir)
    return nc


def _prep_inputs(in_data, Wq, Wk, Wv, Wo):
    import ml_dtypes

    bf16 = ml_dtypes.bfloat16
    in_maps = []
    for c in range(NCORES):
        b, g = c // 4, c % 4
        sl = slice(g * FPC, (g + 1) * FPC)
        in_maps.append({
            "xT": np.ascontiguousarray(in_data[b].T).astype(bf16),
            "wqT": np.ascontiguousarray(Wq[sl, :].T).astype(bf16),
            "wkT": np.ascontiguousarray(Wk[sl, :].T).astype(bf16),
            "wvT": np.ascontiguousarray(Wv[sl, :].T).astype(bf16),
            "woT": np.ascontiguousarray(Wo[:, sl].T).astype(bf16),
        })
    return in_maps


def run(inputs, trace=False):
    from concourse.bass_utils import run_bass_kernel_spmd

    in_data = np.asarray(inputs["in_data"], dtype=np.float32)
    Wq = np.asarray(inputs["Wq"], dtype=np.float32)
    Wk = np.asarray(inputs["Wk"], dtype=np.float32)
    Wv = np.asarray(inputs["Wv"], dtype=np.float32)
    Wo = np.asarray(inputs["Wo"], dtype=np.float32)
    bo = np.asarray(inputs["bo"], dtype=np.float32)

    if "nc" not in _CACHE:
        _CACHE["nc"] = _build()
    nc = _CACHE["nc"]

    in_maps = _prep_inputs(in_data, Wq, Wk, Wv, Wo)
    kw = {}
    if trace:
        kw = dict(trace=True, trace_cores=list(range(NCORES)))
    res = run_bass_kernel_spmd(nc, in_maps, core_ids=list(range(NCORES)), **kw)

    out = np.zeros((B, S, D), dtype=np.float32)
    for c in range(NCORES):
        out[c // 4] += res.results[c]["outp"]
    out += bo[None, None, :]
    return out, res


def kernel(**inputs) -> np.ndarray:
    out, _ = run(inputs)
    return out

